# revision 1
# baseline (speedup 1.0000x reference)
"""GIN (MoMuGNN) message-passing kernel for 8 TRN2 NeuronCores."""

import numpy as np
from dataclasses import dataclass, field

import concourse.bass as bass
import concourse.tile as tile
from concourse import bacc, mybir

P = 128
NC = 8
BN_EPS = 1e-5
F32 = mybir.dt.float32
F16 = mybir.dt.float16


@dataclass
class Cfg:
    N: int
    E: int
    L: int
    G: int
    F: int = 128

    @property
    def npc(self):
        return self.N // NC

    @property
    def half(self):
        return self.N // 2

    @property
    def ntiles(self):
        return (self.npc + P - 1) // P

    def tsize(self, t):
        return min(P, self.npc - t * P)

    @property
    def groups(self):
        gs = []
        t = 0
        while t < self.ntiles:
            gs.append(list(range(t, min(t + 4, self.ntiles))))
            t += 4
        return gs


@dataclass
class Sched:
    K: np.ndarray          # [ntiles, 2] chunks per (tile, half), uniform over cores
    idx16: list            # per core: [128, total_chunks*8] int16 wrapped
    dstl: list             # per core: [128, total_chunks] fp32
    chunk_meta: list = field(default_factory=list)  # per chunk (in idx order): (tile, half)
    total_chunks: int = 0


def build_schedule(cfg: Cfg, edge_index: np.ndarray) -> Sched:
    """edge_index [2, E] int. Chunks bucketed per (group, src-half); dst_local
    is group-local (0..gw-1). Within a bucket edges are sorted by src."""
    src = edge_index[0].astype(np.int64)
    dst = edge_index[1].astype(np.int64)
    npc, half = cfg.npc, cfg.half
    groups = cfg.groups
    ngr = len(groups)
    core = dst // npc
    loc = dst % npc
    gi = loc // (4 * P)            # group within core (4 tiles per group)
    dl = loc - gi * 4 * P          # dst local within group
    hf = (src >= half).astype(np.int64)

    buckets = {}
    order = np.lexsort((src, hf, gi, core))
    cs, gs_, hs = core[order], gi[order], hf[order]
    srcs = np.where(hf[order] == 1, src[order] - half, src[order])
    dls = dl[order]
    key = (cs * ngr + gs_) * 2 + hs
    bounds = np.searchsorted(key, np.arange(NC * ngr * 2 + 1))
    cnt = np.zeros((NC, ngr, 2), np.int64)
    for c in range(NC):
        for g in range(ngr):
            for h in range(2):
                k = (c * ngr + g) * 2 + h
                a, b = bounds[k], bounds[k + 1]
                buckets[(c, g, h)] = (srcs[a:b], dls[a:b])
                cnt[c, g, h] = b - a

    K = np.zeros((ngr, 2), np.int64)
    for g in range(ngr):
        for h in range(2):
            m = cnt[:, g, h].max()
            K[g, h] = (m + P - 1) // P if m > 0 else 0
        if K[g].sum() == 0:
            K[g, 0] = 1

    chunk_meta = []
    for g in range(ngr):
        for h in range(2):
            chunk_meta.extend([(g, h)] * int(K[g, h]))
    total_chunks = len(chunk_meta)

    idx16, dstl = [], []
    for c in range(NC):
        flat_idx = np.zeros(total_chunks * P, np.uint16)
        flat_dl = np.full((P, total_chunks), -1.0, np.float32)
        pos = 0
        for g in range(ngr):
            for h in range(2):
                k = int(K[g, h])
                if k == 0:
                    continue
                sarr, darr = buckets[(c, g, h)]
                n = len(sarr)
                padded_s = np.zeros(k * P, np.uint16)
                padded_s[:n] = sarr.astype(np.uint16)
                flat_idx[pos * P:(pos + k) * P] = padded_s
                dcol = np.full(k * P, -1.0, np.float32)
                dcol[:n] = darr.astype(np.float32)
                flat_dl[:, pos:pos + k] = dcol.reshape(k, P).T
                pos += k
        assert pos == total_chunks
        w = np.zeros((16, total_chunks * 8), np.uint16)
        fi = flat_idx.reshape(total_chunks * 8, 16)  # i = s*16 + p
        w[:, :] = fi.T
        idx16.append(np.tile(w, (8, 1)).view(np.int16))
        dstl.append(flat_dl)

    return Sched(K=K, idx16=idx16, dstl=dstl, chunk_meta=chunk_meta,
                 total_chunks=total_chunks)


def build_nc(cfg: Cfg, sched: Sched):
    npc, ntiles, L, N = cfg.npc, cfg.ntiles, cfg.L, cfg.N
    half = cfg.half
    TC = sched.total_chunks
    K = sched.K
    relu_op = mybir.ActivationFunctionType.Relu
    copy_op = mybir.ActivationFunctionType.Copy

    nc = bacc.Bacc("TRN2", target_bir_lowering=False, debug=False, num_devices=NC)

    z0t_d = nc.dram_tensor("z0t", [P, npc], F32, kind="ExternalInput")
    idx_d = nc.dram_tensor("idx16", [P, TC * 8], mybir.dt.int16, kind="ExternalInput")
    dstl_d = nc.dram_tensor("dstl", [P, TC], F32, kind="ExternalInput")
    iota_d = nc.dram_tensor("iota", [P, 4 * P], F32, kind="ExternalInput")
    ident_d = nc.dram_tensor("ident", [P, P], F32, kind="ExternalInput")
    w1_d = nc.dram_tensor("w1", [P, L * 2 * P], F32, kind="ExternalInput")   # [F, l*256+c]
    w2_d = nc.dram_tensor("w2", [P, L * 2 * P], F32, kind="ExternalInput")   # [c-half part, l*2*128+h*128+f]
    b1_d = nc.dram_tensor("b1", [P, L * 2], F32, kind="ExternalInput")       # [c within half, l*2+h]
    b2_d = nc.dram_tensor("b2", [P, L], F32, kind="ExternalInput")
    gam_d = nc.dram_tensor("gam", [P, L], F32, kind="ExternalInput")
    bet_d = nc.dram_tensor("bet", [P, L], F32, kind="ExternalInput")

    h5_out = nc.dram_tensor("h5T", [P, npc], F32, kind="ExternalOutput")

    ag_in = [nc.dram_tensor(f"ag_in_{l}", [npc, P], F16, kind="Internal")
             for l in range(L - 1)]
    ag_out = [nc.dram_tensor(f"ag_out_{l}", [N, P], F16, kind="Internal",
                             addr_space="Shared") for l in range(L - 1)]
    ar_in = [nc.dram_tensor(f"ar_in_{l}", [P, 2], F32, kind="Internal")
             for l in range(L)]
    ar_out = [nc.dram_tensor(f"ar_out_{l}", [P, 2], F32, kind="Internal",
                             addr_space="Shared") for l in range(L)]
    rg = [list(range(NC))]

    inv_n = 1.0 / N

    with tile.TileContext(nc) as tc:
        with tc.tile_pool(name="const", bufs=1) as cp, \
             tc.tile_pool(name="gath", bufs=2) as gp, \
             tc.tile_pool(name="oh", bufs=4) as ohp, \
             tc.tile_pool(name="zn", bufs=3) as znp, \
             tc.tile_pool(name="u", bufs=2) as up, \
             tc.tile_pool(name="small", bufs=8) as sp, \
             tc.tile_pool(name="scr", bufs=2) as scrp, \
             tc.tile_pool(name="ps_agg", bufs=2, space="PSUM") as pagg, \
             tc.tile_pool(name="ps_mlp", bufs=2, space="PSUM") as pmlp, \
             tc.tile_pool(name="ps_tp", bufs=2, space="PSUM") as ptp:

            # ---- persistent SBUF ----
            idx_sb = cp.tile([P, TC * 8], mybir.dt.int16)
            nc.sync.dma_start(out=idx_sb[:], in_=idx_d[:, :])
            dstl_sb = cp.tile([P, TC], F32)
            nc.sync.dma_start(out=dstl_sb[:], in_=dstl_d[:, :])
            iota_sb = cp.tile([P, 4 * P], F32)
            nc.sync.dma_start(out=iota_sb[:], in_=iota_d[:, :])
            ident_sb = cp.tile([P, P], F32)
            nc.sync.dma_start(out=ident_sb[:], in_=ident_d[:, :])
            w1_sb = cp.tile([P, L * 2 * P], F32)
            nc.sync.dma_start(out=w1_sb[:], in_=w1_d[:, :])
            w2_sb = cp.tile([P, L * 2 * P], F32)
            nc.sync.dma_start(out=w2_sb[:], in_=w2_d[:, :])
            b1_sb = cp.tile([P, L * 2], F32)
            nc.sync.dma_start(out=b1_sb[:], in_=b1_d[:, :])
            b2_sb = cp.tile([P, L], F32)
            nc.sync.dma_start(out=b2_sb[:], in_=b2_d[:, :])
            gam_sb = cp.tile([P, L], F32)
            nc.sync.dma_start(out=gam_sb[:], in_=gam_d[:, :])
            bet_sb = cp.tile([P, L], F32)
            nc.sync.dma_start(out=bet_sb[:], in_=bet_d[:, :])

            eps_sb = cp.tile([P, 1], F32)
            nc.vector.memset(eps_sb[:], BN_EPS)
            zero_sb = cp.tile([P, 1], F32)
            nc.vector.memset(zero_sb[:], 0.0)
            z0_sb = cp.tile([P, npc], F32)
            nc.sync.dma_start(out=z0_sb[:], in_=z0t_d[:, :])
            iota16 = cp.tile([P, 4 * P], F16)
            nc.vector.tensor_copy(out=iota16[:], in_=iota_sb[:])
            ident16 = cp.tile([P, P], F16)
            nc.vector.tensor_copy(out=ident16[:], in_=ident_sb[:])
            hrm = [cp.tile([P, ntiles * P], F16, name=f"hrm{i}") for i in range(2)]
            z2all = cp.tile([P, npc], F32)
            nstats = len(cfg.groups)
            ssum = cp.tile([P, nstats], F32)
            ssq = cp.tile([P, nstats], F32)

            for l in range(L):
                table = None if l == 0 else ag_out[l - 1]
                selfbuf = None if l == 0 else hrm[(l - 1) % 2]
                dt_m = F16
                iota_m = iota16
                ident_m = ident16
                last = l == L - 1

                # chunk columns are laid out in group order already
                chunk_pos = 0
                for gi, g in enumerate(cfg.groups):
                    gw = sum(cfg.tsize(t) for t in g)
                    goff = g[0] * P
                    if l == 0:
                        # layer-0 z = x + A@x precomputed on host: skip
                        # gather/aggregation entirely
                        zt = z0_sb[:, goff:goff + gw]
                        u_t = [up.tile([P, gw], F32, name=f"u{hh}", tag=f"u{hh}",
                                       padded_shape=[P, 4 * P]) for hh in range(2)]
                        for hh in range(2):
                            ps1 = pmlp.tile([P, gw], F32, name="ps1", tag="ps1",
                                            padded_shape=[P, 4 * P], space="PSUM")
                            nc.tensor.matmul(
                                out=ps1[:, :],
                                lhsT=w1_sb[:, l * 2 * P + hh * P:l * 2 * P + hh * P + P],
                                rhs=zt,
                                start=True, stop=True)
                            nc.scalar.activation(
                                out=u_t[hh][:, :], in_=ps1[:, :], func=relu_op,
                                bias=b1_sb[:, l * 2 + hh:l * 2 + hh + 1], scale=1.0)
                        ps2 = pmlp.tile([P, gw], F32, name="ps2", tag="ps2",
                                        padded_shape=[P, 4 * P], space="PSUM")
                        for hh in range(2):
                            nc.tensor.matmul(
                                out=ps2[:, :],
                                lhsT=w2_sb[:, (l * 2 + hh) * P:(l * 2 + hh) * P + P],
                                rhs=u_t[hh][:, :],
                                start=(hh == 0), stop=(hh == 1))
                        nc.vector.tensor_scalar(
                            out=z2all[:, goff:goff + gw], in0=ps2[:, :],
                            scalar1=b2_sb[:, l:l + 1], scalar2=None,
                            op0=mybir.AluOpType.add)
                        nc.vector.tensor_reduce(
                            out=ssum[:, gi:gi + 1], in_=z2all[:, goff:goff + gw],
                            axis=mybir.AxisListType.X, op=mybir.AluOpType.add)
                        sq_scr = scrp.tile([P, 4 * P], F32, name="sq_scr", tag="sq")
                        nc.scalar.activation(
                            out=sq_scr[:, 0:gw], in_=z2all[:, goff:goff + gw],
                            func=mybir.ActivationFunctionType.Square,
                            bias=zero_sb[:, 0:1],
                            accum_out=ssq[:, gi:gi + 1])
                        continue
                    klo = int(K[gi, 0])
                    khi = int(K[gi, 1])
                    kg = klo + khi
                    gt = gp.tile([P, kg * P], dt_m, name="gt", tag="gt")
                    if klo:
                        nc.gpsimd.dma_gather(
                            gt[:, :klo * P].rearrange("p (c f) -> p c f", f=P),
                            table[0:half, :],
                            idx_sb[:, chunk_pos * 8:(chunk_pos + klo) * 8],
                            klo * P, klo * P, P, elem_step=P, single_packet=False)
                    if khi:
                        nc.gpsimd.dma_gather(
                            gt[:, klo * P:kg * P].rearrange("p (c f) -> p c f", f=P),
                            table[half:N, :],
                            idx_sb[:, (chunk_pos + klo) * 8:(chunk_pos + kg) * 8],
                            khi * P, khi * P, P, elem_step=P, single_packet=False)

                    psum = pagg.tile([P, gw], F32, name="psum", tag="psum",
                                     padded_shape=[P, 4 * P], space="PSUM")
                    # one PSUM accumulation group per psum tile:
                    # self matmuls first (start on the very first), then
                    # group-wide chunk matmuls, stop on the last chunk.
                    toff = 0
                    for ti, t in enumerate(g):
                        ts_ = cfg.tsize(t)
                        nc.tensor.matmul(
                            out=psum[:, toff:toff + ts_],
                            lhsT=selfbuf[0:ts_, t * P:t * P + P],
                            rhs=ident_m[0:ts_, 0:ts_],
                            start=(ti == 0), stop=False)
                        toff += ts_
                    for j in range(kg):
                        oh = ohp.tile([P, 4 * P], dt_m, name="oh", tag="oh")
                        nc.vector.tensor_scalar(
                            out=oh[:, 0:gw], in0=iota_m[:, 0:gw],
                            scalar1=dstl_sb[:, chunk_pos + j:chunk_pos + j + 1],
                            scalar2=None, op0=mybir.AluOpType.is_equal)
                        nc.tensor.matmul(
                            out=psum[:, 0:gw],
                            lhsT=gt[:, j * P:(j + 1) * P],
                            rhs=oh[:, 0:gw],
                            start=False, stop=(j == kg - 1))
                    chunk_pos += kg

                    # ---- MLP ----
                    goff = g[0] * P  # start column of group in z/zT buffers
                    zt = up.tile([P, gw], F32, name="zt", tag="zt",
                                 padded_shape=[P, 4 * P])
                    nc.vector.tensor_copy(out=zt[:, :], in_=psum[:, :])
                    u_t = [up.tile([P, gw], F32, name=f"u{hh}", tag=f"u{hh}",
                                   padded_shape=[P, 4 * P]) for hh in range(2)]
                    for hh in range(2):
                        ps1 = pmlp.tile([P, gw], F32, name="ps1", tag="ps1",
                                        padded_shape=[P, 4 * P], space="PSUM")
                        nc.tensor.matmul(
                            out=ps1[:, :],
                            lhsT=w1_sb[:, l * 2 * P + hh * P:l * 2 * P + hh * P + P],
                            rhs=zt[:, :],
                            start=True, stop=True)
                        nc.scalar.activation(
                            out=u_t[hh][:, :], in_=ps1[:, :], func=relu_op,
                            bias=b1_sb[:, l * 2 + hh:l * 2 + hh + 1], scale=1.0)
                    ps2 = pmlp.tile([P, gw], F32, name="ps2", tag="ps2",
                                    padded_shape=[P, 4 * P], space="PSUM")
                    for hh in range(2):
                        nc.tensor.matmul(
                            out=ps2[:, :],
                            lhsT=w2_sb[:, (l * 2 + hh) * P:(l * 2 + hh) * P + P],
                            rhs=u_t[hh][:, :],
                            start=(hh == 0), stop=(hh == 1))
                    # z2 = ps2 + b2 -> z2all slice
                    nc.vector.tensor_scalar(
                        out=z2all[:, goff:goff + gw], in0=ps2[:, :],
                        scalar1=b2_sb[:, l:l + 1], scalar2=None,
                        op0=mybir.AluOpType.add)
                    # stats
                    nc.vector.tensor_reduce(
                        out=ssum[:, gi:gi + 1], in_=z2all[:, goff:goff + gw],
                        axis=mybir.AxisListType.X, op=mybir.AluOpType.add)
                    sq_scr = scrp.tile([P, 4 * P], F32, name="sq_scr", tag="sq")
                    nc.scalar.activation(
                        out=sq_scr[:, 0:gw], in_=z2all[:, goff:goff + gw],
                        func=mybir.ActivationFunctionType.Square,
                        bias=zero_sb[:, 0:1],
                        accum_out=ssq[:, gi:gi + 1])

                # ---- BN stats allreduce ----
                ar_sb = sp.tile([P, 2], F32, name="ar_sb", tag="ar")
                nc.vector.tensor_reduce(out=ar_sb[:, 0:1], in_=ssum[:, :],
                                        axis=mybir.AxisListType.X,
                                        op=mybir.AluOpType.add)
                nc.vector.tensor_reduce(out=ar_sb[:, 1:2], in_=ssq[:, :],
                                        axis=mybir.AxisListType.X,
                                        op=mybir.AluOpType.add)
                nc.sync.dma_start(out=ar_in[l][:, :], in_=ar_sb[:, :])
                nc.gpsimd.collective_compute(
                    "AllReduce", mybir.AluOpType.add, replica_groups=rg,
                    ins=[ar_in[l][:, :]], outs=[ar_out[l][:, :]])
                arr = sp.tile([P, 2], F32, name="arr", tag="ar")
                nc.sync.dma_start(out=arr[:, :], in_=ar_out[l][:, :])

                stat = sp.tile([P, 6], F32, name="stat", tag="stat")
                mean, msq, var, istd, s_col, t_col = [stat[:, i:i + 1] for i in range(6)]
                nc.vector.tensor_scalar(out=mean, in0=arr[:, 0:1], scalar1=inv_n,
                                        scalar2=None, op0=mybir.AluOpType.mult)
                nc.vector.tensor_scalar(out=msq, in0=arr[:, 1:2], scalar1=inv_n,
                                        scalar2=None, op0=mybir.AluOpType.mult)
                # var = msq - mean^2
                sq_t = sp.tile([P, 2], F32, name="sq_t", tag="sq_t")
                nc.vector.tensor_tensor(out=sq_t[:, 0:1], in0=mean, in1=mean,
                                        op=mybir.AluOpType.mult)
                nc.vector.tensor_tensor(out=var, in0=msq, in1=sq_t[:, 0:1],
                                        op=mybir.AluOpType.subtract)
                std_t = sp.tile([P, 2], F32, name="std_t", tag="sq_t")
                nc.scalar.activation(out=std_t[:, 0:1], in_=var,
                                     func=mybir.ActivationFunctionType.Sqrt,
                                     bias=eps_sb[:, 0:1], scale=1.0)
                nc.vector.reciprocal(out=istd, in_=std_t[:, 0:1])
                nc.vector.tensor_tensor(out=s_col, in0=gam_sb[:, l:l + 1], in1=istd,
                                        op=mybir.AluOpType.mult)
                nc.vector.tensor_tensor(out=sq_t[:, 1:2], in0=mean, in1=s_col,
                                        op=mybir.AluOpType.mult)
                nc.vector.tensor_tensor(out=t_col, in0=bet_sb[:, l:l + 1],
                                        in1=sq_t[:, 1:2],
                                        op=mybir.AluOpType.subtract)

                # ---- normalize (+relu except last) ----
                act = copy_op if last else relu_op
                if last:
                    for gi2, g in enumerate(cfg.groups):
                        goff = g[0] * P
                        gw = sum(cfg.tsize(t) for t in g)
                        zn = znp.tile([P, 4 * P], F32, name="zn", tag="zn")
                        nc.vector.tensor_scalar(
                            out=zn[:, 0:gw], in0=z2all[:, goff:goff + gw],
                            scalar1=s_col, scalar2=t_col,
                            op0=mybir.AluOpType.mult, op1=mybir.AluOpType.add)
                        nc.sync.dma_start(out=h5_out[:, goff:goff + gw],
                                          in_=zn[:, 0:gw])
                else:
                    hout = hrm[l % 2]
                    for t in range(ntiles):
                        ts_ = cfg.tsize(t)
                        zn = znp.tile([P, 4 * P], F16, name="zn16", tag="zn16")
                        nc.scalar.activation(out=zn[:, 0:ts_],
                                             in_=z2all[:, t * P:t * P + ts_],
                                             func=act, bias=t_col, scale=s_col)
                        tp = ptp.tile([P, P], F16, name="tp", tag="tp",
                                      space="PSUM")
                        nc.tensor.transpose(out=tp[0:ts_, :], in_=zn[:, 0:ts_],
                                            identity=ident16[:, :])
                        nc.vector.tensor_copy(out=hout[0:ts_, t * P:t * P + P],
                                              in_=tp[0:ts_, :])
                    # DMA h_rm -> ag_in (row-major [npc, 128])
                    nfull = npc // P
                    if nfull:
                        nc.sync.dma_start(
                            out=ag_in[l][0:nfull * P, :].rearrange(
                                "(t p) f -> p t f", p=P),
                            in_=hout[:, 0:nfull * P].rearrange(
                                "p (t f) -> p t f", f=P))
                    if npc % P:
                        ts_ = npc % P
                        nc.sync.dma_start(
                            out=ag_in[l][nfull * P:npc, :],
                            in_=hout[0:ts_, nfull * P:nfull * P + P])
                    nc.gpsimd.collective_compute(
                        "AllGather", mybir.AluOpType.bypass, replica_groups=rg,
                        ins=[ag_in[l][:, :]], outs=[ag_out[l][:, :]])

    nc.compile()
    return nc


def prep_inputs(cfg: Cfg, sched: Sched, x, W1, b1, W2, b2, gamma, beta,
                edge_index):
    """Build per-core in_maps (numpy). Layer-0 z = x + A@x is host-computed."""
    N, L, ntiles, npc = cfg.N, cfg.L, cfg.ntiles, cfg.npc
    x = np.asarray(x, np.float32)
    src = np.asarray(edge_index[0], np.int64)
    dst = np.asarray(edge_index[1], np.int64)
    try:
        import jax
        with jax.default_device(jax.devices("cpu")[0]):
            agg0 = np.asarray(jax.ops.segment_sum(x[src], dst, num_segments=N))
    except Exception:
        agg0 = np.zeros_like(x)
        np.add.at(agg0, dst, x[src])
    z0 = x + agg0
    iota = np.broadcast_to(np.arange(4 * P, dtype=np.float32), (P, 4 * P)).copy()
    ident = np.eye(P, dtype=np.float32)
    w1 = np.ascontiguousarray(np.transpose(np.asarray(W1, np.float32), (1, 0, 2))
                              ).reshape(P, L * 2 * P)
    w2 = np.ascontiguousarray(np.transpose(
        np.asarray(W2, np.float32).reshape(L, 2, P, P), (2, 0, 1, 3))
        ).reshape(P, L * 2 * P)
    b1r = np.ascontiguousarray(np.transpose(
        np.asarray(b1, np.float32).reshape(L, 2, P), (2, 0, 1))).reshape(P, L * 2)
    b2r = np.ascontiguousarray(np.asarray(b2, np.float32).T)  # [128, L]
    gam = np.ascontiguousarray(np.asarray(gamma, np.float32).T)
    bet = np.ascontiguousarray(np.asarray(beta, np.float32).T)

    in_maps = []
    for c in range(NC):
        xs = np.ascontiguousarray(z0[c * npc:(c + 1) * npc].T)  # [F, npc]
        in_maps.append({
            "z0t": xs,
            "idx16": sched.idx16[c], "dstl": sched.dstl[c],
            "iota": iota, "ident": ident,
            "w1": w1, "w2": w2, "b1": b1r, "b2": b2r, "gam": gam, "bet": bet,
        })
    return in_maps


def combine_outputs(cfg: Cfg, results, batch, num_graphs):
    """results: list of per-core dicts with h5T [128, npc]. Host segment-max."""
    h5 = np.concatenate([r["h5T"] for r in results], axis=1).T  # [N, F]
    h5 = h5[:cfg.N]
    batch = np.asarray(batch)
    G = int(num_graphs)
    out = np.full((G, cfg.F), -np.inf, np.float32)
    starts = np.searchsorted(batch, np.arange(G))
    ends = np.searchsorted(batch, np.arange(G), side="right")
    ends = np.searchsorted(batch, np.arange(1, G + 1))
    for g in range(G):
        if ends[g] > starts[g]:
            out[g] = h5[starts[g]:ends[g]].max(axis=0)
    return out

# ---------------------------------------------------------------------------
# Harness entry point
# ---------------------------------------------------------------------------
import hashlib

_CACHE = {}


def kernel(x, edge_index, batch, num_graphs, W1, b1, W2, b2, gamma, beta):
    """GIN forward on 8 TRN2 NeuronCores. Full inputs in, full output out."""
    from concourse.bass_utils import run_bass_kernel_spmd

    x = np.asarray(x, np.float32)
    edge_index = np.asarray(edge_index)
    batch = np.asarray(batch)
    W1 = np.asarray(W1, np.float32)
    b1 = np.asarray(b1, np.float32)
    W2 = np.asarray(W2, np.float32)
    b2 = np.asarray(b2, np.float32)
    gamma = np.asarray(gamma, np.float32)
    beta = np.asarray(beta, np.float32)
    G = int(np.asarray(num_graphs))

    cfg = Cfg(N=x.shape[0], E=edge_index.shape[1], L=W1.shape[0], G=G)
    key = (x.shape, edge_index.shape, cfg.L,
           hashlib.blake2b(np.ascontiguousarray(edge_index).tobytes(),
                           digest_size=16).hexdigest())
    if key not in _CACHE:
        sched = build_schedule(cfg, edge_index)
        nc = build_nc(cfg, sched)
        _CACHE[key] = (sched, nc)
    sched, nc = _CACHE[key]

    in_maps = prep_inputs(cfg, sched, x, W1, b1, W2, b2, gamma, beta, edge_index)
    res = run_bass_kernel_spmd(nc, in_maps, core_ids=list(range(NC)))
    return combine_outputs(cfg, res.results, batch, G)



# revision 5
# speedup vs baseline: 757.4282x; 757.4282x over previous
"""GIN (MoMuGNN) message-passing kernel for 8 TRN2 NeuronCores.

Full inputs in, full output out. All graph compute runs on device:
per-layer edge gather (SWDGE), one-hot scatter-add matmuls into PSUM,
MLP, batch-norm (stats via AllReduce), inter-layer fp16 AllGather of
node features, and the final per-graph segment-max (transpose-gather +
max reduces + AllReduce-max). Host work is limited to data layout
(transpose/cast/shard) and edge-schedule construction, cached per graph.
"""

import hashlib
import numpy as np
from dataclasses import dataclass, field

import concourse.bass as bass
import concourse.tile as tile
from concourse import bacc, mybir

P = 128
NC = 8
BN_EPS = 1e-5
F32 = mybir.dt.float32
F16 = mybir.dt.float16


@dataclass
class Cfg:
    N: int
    E: int
    L: int
    G: int
    F: int = 128

    @property
    def npc(self):
        return self.N // NC

    @property
    def half(self):
        return self.N // 2

    @property
    def ntiles(self):
        return (self.npc + P - 1) // P

    def tsize(self, t):
        return min(P, self.npc - t * P)

    @property
    def groups(self):
        gs = []
        t = 0
        while t < self.ntiles:
            gs.append(list(range(t, min(t + 4, self.ntiles))))
            t += 4
        return gs


def _wrap_idx16(flat_idx: np.ndarray, nchunks: int) -> np.ndarray:
    """[nchunks*128] uint16 -> [128, nchunks*8] int16 in the wrapped layout
    dma_gather expects (16-partition wrap, replicated to 128)."""
    w16 = np.zeros((16, nchunks * 8), np.uint16)
    fi = flat_idx.reshape(nchunks * 8, 16)
    w16[:, :] = fi.T
    return np.tile(w16, (8, 1)).view(np.int16)


@dataclass
class Sched:
    K: np.ndarray          # [ntiles, 2] chunks per (window-tile, half), max over cores
    idx16: list            # per core: [128, total_chunks*8] int16 wrapped
    dstl: list             # per core: [128, total_chunks] fp32 (window-local dst, -1 pad)
    chunk_meta: list = field(default_factory=list)  # per chunk: (tile, half)
    total_chunks: int = 0
    # ---- segment-max schedule ----
    gm_reduces: list = field(default_factory=list)  # (graph, chunk_off, nchunks)
    gm_batches: list = field(default_factory=list)  # chunks per gather call
    gm_idx16: list = field(default_factory=list)    # per core: [128, gm_total*8]
    gm_total: int = 0


def build_schedule(cfg: Cfg, edge_index: np.ndarray, batch: np.ndarray) -> Sched:
    """Bucket edges per (dst-core, 128-dst window, src-half); dst_local is
    window-local (0..127). Within a bucket edges are sorted by src so the
    gather walks ascending HBM addresses. Also builds the segment-max
    gather schedule (uniform across cores; per-core index data)."""
    src = edge_index[0].astype(np.int64)
    dst = edge_index[1].astype(np.int64)
    npc, half, ntiles = cfg.npc, cfg.half, cfg.ntiles
    core = dst // npc
    loc = dst % npc
    wi = loc // P                  # window (=tile) within core
    dl = loc - wi * P              # dst local within window
    hf = (src >= half).astype(np.int64)

    order = np.lexsort((src, hf, wi, core))
    cs, ws_, hs = core[order], wi[order], hf[order]
    srcs = np.where(hs == 1, src[order] - half, src[order])
    dls = dl[order]
    key = (cs * ntiles + ws_) * 2 + hs
    bounds = np.searchsorted(key, np.arange(NC * ntiles * 2 + 1))
    buckets = {}
    cnt = np.zeros((NC, ntiles, 2), np.int64)
    for c in range(NC):
        for w in range(ntiles):
            for h in range(2):
                k = (c * ntiles + w) * 2 + h
                a, b = bounds[k], bounds[k + 1]
                buckets[(c, w, h)] = (srcs[a:b], dls[a:b])
                cnt[c, w, h] = b - a

    K = np.zeros((ntiles, 2), np.int64)
    for w in range(ntiles):
        for h in range(2):
            m = cnt[:, w, h].max()
            K[w, h] = (m + P - 1) // P if m > 0 else 0
    # guarantee each 4-window group issues at least one chunk so the PSUM
    # accumulation group always has a stop matmul
    for g in cfg.groups:
        if sum(int(K[w, h]) for w in g for h in range(2)) == 0:
            K[g[0], 0] = 1

    # chunk order per group: halves outer (matches the two gather calls),
    # windows inner
    chunk_meta = []
    for g in cfg.groups:
        for h in range(2):
            for w in g:
                chunk_meta.extend([(w, h)] * int(K[w, h]))
    total_chunks = len(chunk_meta)

    idx16, dstl = [], []
    for c in range(NC):
        flat_idx = np.zeros(total_chunks * P, np.uint16)
        flat_dl = np.full((P, total_chunks), -1.0, np.float32)
        pos = 0
        for g in cfg.groups:
            for h in range(2):
                for w in g:
                    k = int(K[w, h])
                    if k == 0:
                        continue
                    sarr, darr = buckets[(c, w, h)]
                    n = len(sarr)
                    padded_s = np.zeros(k * P, np.uint16)
                    padded_s[:n] = sarr.astype(np.uint16)
                    flat_idx[pos * P:(pos + k) * P] = padded_s
                    dcol = np.full(k * P, -1.0, np.float32)
                    dcol[:n] = darr.astype(np.float32)
                    flat_dl[:, pos:pos + k] = dcol.reshape(k, P).T
                    pos += k
        assert pos == total_chunks
        idx16.append(_wrap_idx16(flat_idx, total_chunks))
        dstl.append(flat_dl)

    # ---- segment-max gather schedule --------------------------------------
    # Per graph g, per core c: local node rows [a, b). Chunk count
    # C_g = max_c ceil(n_cg/128) (uniform). Cores pad with duplicates of a
    # local row of g, or the -inf sentinel row (npc) when they own none.
    batch = np.asarray(batch, np.int64)
    G = cfg.G
    starts = np.searchsorted(batch, np.arange(G))
    ends = np.searchsorted(batch, np.arange(1, G + 1))
    spans = []      # per graph: per core (a, b) local rows
    Cg = np.zeros(G, np.int64)
    for g in range(G):
        row = []
        for c in range(NC):
            c0, c1 = c * npc, (c + 1) * npc
            a, b = max(int(starts[g]), c0), min(int(ends[g]), c1)
            row.append((a - c0, b - c0) if b > a else (0, 0))
        spans.append(row)
        Cg[g] = max((b - a + P - 1) // P for a, b in row)

    gm_reduces = []
    off = 0
    for g in range(G):
        if Cg[g] > 0:
            gm_reduces.append((g, off, int(Cg[g])))
            off += int(Cg[g])
    gm_total = off

    # batches: split gather calls at graph boundaries, <= 64 chunks per call
    gm_batches = []
    cur = 0
    for g, o, c in gm_reduces:
        if cur and cur + c > 64:
            gm_batches.append(cur)
            cur = 0
        cur += c
    if cur:
        gm_batches.append(cur)

    sentinel = npc  # z5T row holding -inf
    gm_idx16 = []
    for c in range(NC):
        flat = np.full(gm_total * P, sentinel, np.uint16)
        for g, o, nch in gm_reduces:
            a, b = spans[g][c]
            n = b - a
            if n == 0:
                continue
            ar = np.arange(o * P, o * P + nch * P)
            vals = np.full(nch * P, a, np.uint16)  # pad with first local row
            vals[:n] = np.arange(a, b, dtype=np.uint16)
            flat[ar] = vals
        gm_idx16.append(_wrap_idx16(flat, gm_total))

    return Sched(K=K, idx16=idx16, dstl=dstl, chunk_meta=chunk_meta,
                 total_chunks=total_chunks, gm_reduces=gm_reduces,
                 gm_batches=gm_batches, gm_idx16=gm_idx16, gm_total=gm_total)


def build_nc(cfg: Cfg, sched: Sched):
    npc, ntiles, L, N, G = cfg.npc, cfg.ntiles, cfg.L, cfg.N, cfg.G
    half = cfg.half
    TC = sched.total_chunks
    GMC = sched.gm_total
    K = sched.K
    relu_op = mybir.ActivationFunctionType.Relu
    copy_op = mybir.ActivationFunctionType.Copy

    nc = bacc.Bacc("TRN2", target_bir_lowering=False, debug=False, num_devices=NC)

    xh_d = nc.dram_tensor("x_hrm", [P, ntiles * P], F16, kind="ExternalInput")
    xt_d = nc.dram_tensor("x_tab", [N, P], F16, kind="ExternalInput")
    idx_d = nc.dram_tensor("idx16", [P, TC * 8], mybir.dt.int16, kind="ExternalInput")
    gmidx_d = nc.dram_tensor("gmidx", [P, GMC * 8], mybir.dt.int16,
                             kind="ExternalInput")
    dstl_d = nc.dram_tensor("dstl", [P, TC], F32, kind="ExternalInput")
    iota_d = nc.dram_tensor("iota", [P, P], F32, kind="ExternalInput")
    ident_d = nc.dram_tensor("ident", [P, P], F32, kind="ExternalInput")
    w1_d = nc.dram_tensor("w1", [P, L * 2 * P], F32, kind="ExternalInput")
    w2_d = nc.dram_tensor("w2", [P, L * 2 * P], F32, kind="ExternalInput")
    b1_d = nc.dram_tensor("b1", [P, L * 2], F32, kind="ExternalInput")
    b2_d = nc.dram_tensor("b2", [P, L], F32, kind="ExternalInput")
    gam_d = nc.dram_tensor("gam", [P, L], F32, kind="ExternalInput")
    bet_d = nc.dram_tensor("bet", [P, L], F32, kind="ExternalInput")

    gmax_out = nc.dram_tensor("gmaxT", [P, G], F32, kind="ExternalOutput")

    ag_in = [nc.dram_tensor(f"ag_in_{l}", [npc, P], F16, kind="Internal")
             for l in range(L - 1)]
    ag_out = [nc.dram_tensor(f"ag_out_{l}", [N, P], F16, kind="Internal",
                             addr_space="Shared") for l in range(L - 1)]
    z5t_d = nc.dram_tensor("z5t", [npc + P, P], F16, kind="Internal")
    ar_in = [nc.dram_tensor(f"ar_in_{l}", [P, 2], F32, kind="Internal")
             for l in range(L)]
    ar_out = [nc.dram_tensor(f"ar_out_{l}", [P, 2], F32, kind="Internal",
                             addr_space="Shared") for l in range(L)]
    gm_in = nc.dram_tensor("gm_in", [P, G], F32, kind="Internal")
    gm_out = nc.dram_tensor("gm_out", [P, G], F32, kind="Internal",
                            addr_space="Shared")
    rg = [list(range(NC))]

    inv_n = 1.0 / N

    with tile.TileContext(nc) as tc:
        with tc.tile_pool(name="const", bufs=1) as cp, \
             tc.tile_pool(name="gath", bufs=2) as gp, \
             tc.tile_pool(name="oh", bufs=4) as ohp, \
             tc.tile_pool(name="zn", bufs=3) as znp, \
             tc.tile_pool(name="u", bufs=2) as up, \
             tc.tile_pool(name="small", bufs=8) as sp, \
             tc.tile_pool(name="scr", bufs=2) as scrp, \
             tc.tile_pool(name="gm", bufs=2) as gmp, \
             tc.tile_pool(name="ps_agg", bufs=2, space="PSUM") as pagg, \
             tc.tile_pool(name="ps_mlp", bufs=2, space="PSUM") as pmlp, \
             tc.tile_pool(name="ps_tp", bufs=2, space="PSUM") as ptp:

            # ---- persistent SBUF ----
            idx_sb = cp.tile([P, TC * 8], mybir.dt.int16)
            nc.sync.dma_start(out=idx_sb[:], in_=idx_d[:, :])
            gmidx_sb = cp.tile([P, GMC * 8], mybir.dt.int16)
            nc.sync.dma_start(out=gmidx_sb[:], in_=gmidx_d[:, :])
            dstl_sb = cp.tile([P, TC], F32)
            nc.sync.dma_start(out=dstl_sb[:], in_=dstl_d[:, :])
            iota_sb = cp.tile([P, P], F32)
            nc.sync.dma_start(out=iota_sb[:], in_=iota_d[:, :])
            ident_sb = cp.tile([P, P], F32)
            nc.sync.dma_start(out=ident_sb[:], in_=ident_d[:, :])
            w1_sb = cp.tile([P, L * 2 * P], F32)
            nc.sync.dma_start(out=w1_sb[:], in_=w1_d[:, :])
            w2_sb = cp.tile([P, L * 2 * P], F32)
            nc.sync.dma_start(out=w2_sb[:], in_=w2_d[:, :])
            b1_sb = cp.tile([P, L * 2], F32)
            nc.sync.dma_start(out=b1_sb[:], in_=b1_d[:, :])
            b2_sb = cp.tile([P, L], F32)
            nc.sync.dma_start(out=b2_sb[:], in_=b2_d[:, :])
            gam_sb = cp.tile([P, L], F32)
            nc.sync.dma_start(out=gam_sb[:], in_=gam_d[:, :])
            bet_sb = cp.tile([P, L], F32)
            nc.sync.dma_start(out=bet_sb[:], in_=bet_d[:, :])

            eps_sb = cp.tile([P, 1], F32)
            nc.vector.memset(eps_sb[:], BN_EPS)
            zero_sb = cp.tile([P, 1], F32)
            nc.vector.memset(zero_sb[:], 0.0)
            ninf_sb = cp.tile([P, P], F16)
            nc.vector.memset(ninf_sb[:], -60000.0)
            iota16 = cp.tile([P, P], F16)
            nc.vector.tensor_copy(out=iota16[:], in_=iota_sb[:])
            ident16 = cp.tile([P, P], F16)
            nc.vector.tensor_copy(out=ident16[:], in_=ident_sb[:])
            hrm = [cp.tile([P, ntiles * P], F16, name=f"hrm{i}") for i in range(2)]
            # hrm[-1 % 2] = x in row-major fp16: layer 0's self term
            nc.sync.dma_start(out=hrm[1][:], in_=xh_d[:, :])
            z2all = cp.tile([P, npc], F32)
            nstats = len(cfg.groups)
            ssum = cp.tile([P, nstats], F32)
            ssq = cp.tile([P, nstats], F32)
            # -inf sentinel rows of z5T (rows npc .. npc+127)
            nc.sync.dma_start(out=z5t_d[npc:npc + P, :], in_=ninf_sb[:, :])

            for l in range(L):
                table = xt_d if l == 0 else ag_out[l - 1]
                selfbuf = hrm[(l - 1) % 2]
                last = l == L - 1

                chunk_pos = 0
                for gi, g in enumerate(cfg.groups):
                    gw = sum(cfg.tsize(t) for t in g)
                    goff = g[0] * P
                    klo = sum(int(K[w, 0]) for w in g)
                    khi = sum(int(K[w, 1]) for w in g)
                    kg = klo + khi
                    gt = gp.tile([P, kg * P], F16, name="gt", tag="gt")
                    if klo:
                        nc.gpsimd.dma_gather(
                            gt[:, :klo * P].rearrange("p (c f) -> p c f", f=P),
                            table[0:half, :],
                            idx_sb[:, chunk_pos * 8:(chunk_pos + klo) * 8],
                            klo * P, klo * P, P, elem_step=P, single_packet=False)
                    if khi:
                        nc.gpsimd.dma_gather(
                            gt[:, klo * P:kg * P].rearrange("p (c f) -> p c f", f=P),
                            table[half:N, :],
                            idx_sb[:, (chunk_pos + klo) * 8:(chunk_pos + kg) * 8],
                            khi * P, khi * P, P, elem_step=P, single_packet=False)

                    psum = pagg.tile([P, gw], F32, name="psum", tag="psum",
                                     padded_shape=[P, 4 * P], space="PSUM")
                    # one PSUM accumulation group: self matmuls first (start
                    # on the very first), then window-local chunk matmuls,
                    # stop on the last chunk.
                    toff = 0
                    for ti, t in enumerate(g):
                        ts_ = cfg.tsize(t)
                        nc.tensor.matmul(
                            out=psum[:, toff:toff + ts_],
                            lhsT=selfbuf[0:ts_, t * P:t * P + P],
                            rhs=ident16[0:ts_, 0:ts_],
                            start=(ti == 0), stop=False)
                        toff += ts_
                    for j in range(kg):
                        w, _h = sched.chunk_meta[chunk_pos + j]
                        ts_ = cfg.tsize(w)
                        woff = (w - g[0]) * P
                        oh = ohp.tile([P, P], F16, name="oh", tag="oh")
                        nc.vector.tensor_scalar(
                            out=oh[:, 0:ts_], in0=iota16[:, 0:ts_],
                            scalar1=dstl_sb[:, chunk_pos + j:chunk_pos + j + 1],
                            scalar2=None, op0=mybir.AluOpType.is_equal)
                        nc.tensor.matmul(
                            out=psum[:, woff:woff + ts_],
                            lhsT=gt[:, j * P:(j + 1) * P],
                            rhs=oh[:, 0:ts_],
                            start=False, stop=(j == kg - 1))
                    chunk_pos += kg

                    # ---- MLP ----
                    zt = up.tile([P, gw], F32, name="zt", tag="zt",
                                 padded_shape=[P, 4 * P])
                    nc.scalar.activation(out=zt[:, :], in_=psum[:, :],
                                         func=copy_op, bias=0.0, scale=1.0)
                    u_t = [up.tile([P, gw], F32, name=f"u{hh}", tag=f"u{hh}",
                                   padded_shape=[P, 4 * P]) for hh in range(2)]
                    for hh in range(2):
                        ps1 = pmlp.tile([P, gw], F32, name="ps1", tag="ps1",
                                        padded_shape=[P, 4 * P], space="PSUM")
                        nc.tensor.matmul(
                            out=ps1[:, :],
                            lhsT=w1_sb[:, l * 2 * P + hh * P:l * 2 * P + hh * P + P],
                            rhs=zt[:, :],
                            start=True, stop=True)
                        nc.scalar.activation(
                            out=u_t[hh][:, :], in_=ps1[:, :], func=relu_op,
                            bias=b1_sb[:, l * 2 + hh:l * 2 + hh + 1], scale=1.0)
                    ps2 = pmlp.tile([P, gw], F32, name="ps2", tag="ps2",
                                    padded_shape=[P, 4 * P], space="PSUM")
                    for hh in range(2):
                        nc.tensor.matmul(
                            out=ps2[:, :],
                            lhsT=w2_sb[:, (l * 2 + hh) * P:(l * 2 + hh) * P + P],
                            rhs=u_t[hh][:, :],
                            start=(hh == 0), stop=(hh == 1))
                    nc.vector.tensor_scalar(
                        out=z2all[:, goff:goff + gw], in0=ps2[:, :],
                        scalar1=b2_sb[:, l:l + 1], scalar2=None,
                        op0=mybir.AluOpType.add)
                    nc.vector.tensor_reduce(
                        out=ssum[:, gi:gi + 1], in_=z2all[:, goff:goff + gw],
                        axis=mybir.AxisListType.X, op=mybir.AluOpType.add)
                    sq_scr = scrp.tile([P, 4 * P], F32, name="sq_scr", tag="sq")
                    nc.scalar.activation(
                        out=sq_scr[:, 0:gw], in_=z2all[:, goff:goff + gw],
                        func=mybir.ActivationFunctionType.Square,
                        bias=zero_sb[:, 0:1],
                        accum_out=ssq[:, gi:gi + 1])

                # ---- BN stats allreduce ----
                ar_sb = sp.tile([P, 2], F32, name="ar_sb", tag="ar")
                nc.vector.tensor_reduce(out=ar_sb[:, 0:1], in_=ssum[:, :],
                                        axis=mybir.AxisListType.X,
                                        op=mybir.AluOpType.add)
                nc.vector.tensor_reduce(out=ar_sb[:, 1:2], in_=ssq[:, :],
                                        axis=mybir.AxisListType.X,
                                        op=mybir.AluOpType.add)
                nc.sync.dma_start(out=ar_in[l][:, :], in_=ar_sb[:, :])
                nc.gpsimd.collective_compute(
                    "AllReduce", mybir.AluOpType.add, replica_groups=rg,
                    ins=[ar_in[l][:, :]], outs=[ar_out[l][:, :]])
                arr = sp.tile([P, 2], F32, name="arr", tag="ar")
                nc.sync.dma_start(out=arr[:, :], in_=ar_out[l][:, :])

                stat = sp.tile([P, 6], F32, name="stat", tag="stat")
                mean, msq, var, istd, s_col, t_col = [stat[:, i:i + 1] for i in range(6)]
                nc.vector.tensor_scalar(out=mean, in0=arr[:, 0:1], scalar1=inv_n,
                                        scalar2=None, op0=mybir.AluOpType.mult)
                nc.vector.tensor_scalar(out=msq, in0=arr[:, 1:2], scalar1=inv_n,
                                        scalar2=None, op0=mybir.AluOpType.mult)
                sq_t = sp.tile([P, 2], F32, name="sq_t", tag="sq_t")
                nc.vector.tensor_tensor(out=sq_t[:, 0:1], in0=mean, in1=mean,
                                        op=mybir.AluOpType.mult)
                nc.vector.tensor_tensor(out=var, in0=msq, in1=sq_t[:, 0:1],
                                        op=mybir.AluOpType.subtract)
                std_t = sp.tile([P, 2], F32, name="std_t", tag="sq_t")
                nc.scalar.activation(out=std_t[:, 0:1], in_=var,
                                     func=mybir.ActivationFunctionType.Sqrt,
                                     bias=eps_sb[:, 0:1], scale=1.0)
                nc.vector.reciprocal(out=istd, in_=std_t[:, 0:1])
                nc.vector.tensor_tensor(out=s_col, in0=gam_sb[:, l:l + 1], in1=istd,
                                        op=mybir.AluOpType.mult)
                nc.vector.tensor_tensor(out=sq_t[:, 1:2], in0=mean, in1=s_col,
                                        op=mybir.AluOpType.mult)
                nc.vector.tensor_tensor(out=t_col, in0=bet_sb[:, l:l + 1],
                                        in1=sq_t[:, 1:2],
                                        op=mybir.AluOpType.subtract)

                # ---- normalize (+relu except last), transpose to row-major ----
                act = mybir.ActivationFunctionType.Identity if last else relu_op
                hout = hrm[l % 2]
                for t in range(ntiles):
                    ts_ = cfg.tsize(t)
                    zn = znp.tile([P, P], F16, name="zn16", tag="zn16")
                    nc.scalar.activation(out=zn[:, 0:ts_],
                                         in_=z2all[:, t * P:t * P + ts_],
                                         func=act, bias=t_col, scale=s_col)
                    tp = ptp.tile([P, P], F16, name="tp", tag="tp",
                                  space="PSUM")
                    nc.tensor.transpose(out=tp[0:ts_, :], in_=zn[:, 0:ts_],
                                        identity=ident16[:, :])
                    nc.vector.tensor_copy(out=hout[0:ts_, t * P:t * P + P],
                                          in_=tp[0:ts_, :])
                dst = z5t_d if last else ag_in[l]
                nfull = npc // P
                if nfull:
                    nc.sync.dma_start(
                        out=dst[0:nfull * P, :].rearrange("(t p) f -> p t f", p=P),
                        in_=hout[:, 0:nfull * P].rearrange("p (t f) -> p t f", f=P))
                if npc % P:
                    ts_ = npc % P
                    nc.sync.dma_start(
                        out=dst[nfull * P:npc, :],
                        in_=hout[0:ts_, nfull * P:nfull * P + P])
                if not last:
                    nc.gpsimd.collective_compute(
                        "AllGather", mybir.AluOpType.bypass, replica_groups=rg,
                        ins=[ag_in[l][:, :]], outs=[ag_out[l][:, :]])

            # ---- global max pool over graph ids (transpose-gather) ----
            gmax = cp.tile([P, G], F32)
            nc.vector.memset(gmax[:], -3.0e38)
            boff = 0       # chunk offset of current batch
            bi = 0         # reduce index
            for nb in sched.gm_batches:
                gat = gmp.tile([P, nb * P], F16, name="gmgat", tag="gmgat")
                nc.gpsimd.dma_gather(
                    gat[:, :].rearrange("p (o n) -> p o n", o=1),
                    z5t_d[0:npc + P, :],
                    gmidx_sb[:, boff * 8:(boff + nb) * 8],
                    nb * P, nb * P, P, elem_step=P, transpose=True,
                    single_packet=False)
                while bi < len(sched.gm_reduces):
                    g, o, c = sched.gm_reduces[bi]
                    if o + c > boff + nb:
                        break
                    nc.vector.tensor_reduce(
                        out=gmax[:, g:g + 1],
                        in_=gat[:, (o - boff) * P:(o - boff + c) * P],
                        axis=mybir.AxisListType.X, op=mybir.AluOpType.max)
                    bi += 1
                boff += nb
            assert bi == len(sched.gm_reduces)
            nc.sync.dma_start(out=gm_in[:, :], in_=gmax[:, :])
            nc.gpsimd.collective_compute(
                "AllReduce", mybir.AluOpType.max, replica_groups=rg,
                ins=[gm_in[:, :]], outs=[gm_out[:, :]])
            gmax2 = sp.tile([P, G], F32, name="gmax2", tag="gmax2")
            nc.sync.dma_start(out=gmax2[:, :], in_=gm_out[:, :])
            nc.sync.dma_start(out=gmax_out[:, :], in_=gmax2[:, :])

    nc.compile()
    return nc


def prep_inputs(cfg: Cfg, sched: Sched, x, W1, b1, W2, b2, gamma, beta):
    """Per-core input maps. Host does data layout only: transpose/cast/shard."""
    N, L, ntiles, npc = cfg.N, cfg.L, cfg.ntiles, cfg.npc
    x = np.asarray(x, np.float32)
    x16 = np.ascontiguousarray(x.astype(np.float16))       # [N, F] gather table
    iota = np.broadcast_to(np.arange(P, dtype=np.float32), (P, P)).copy()
    ident = np.eye(P, dtype=np.float32)
    w1 = np.ascontiguousarray(np.transpose(np.asarray(W1, np.float32), (1, 0, 2))
                              ).reshape(P, L * 2 * P)
    w2 = np.ascontiguousarray(np.transpose(
        np.asarray(W2, np.float32).reshape(L, 2, P, P), (2, 0, 1, 3))
        ).reshape(P, L * 2 * P)
    b1r = np.ascontiguousarray(np.transpose(
        np.asarray(b1, np.float32).reshape(L, 2, P), (2, 0, 1))).reshape(P, L * 2)
    b2r = np.ascontiguousarray(np.asarray(b2, np.float32).T)
    gam = np.ascontiguousarray(np.asarray(gamma, np.float32).T)
    bet = np.ascontiguousarray(np.asarray(beta, np.float32).T)

    in_maps = []
    for c in range(NC):
        shard = x16[c * npc:(c + 1) * npc]                 # [npc, F]
        xh = np.zeros((P, ntiles * P), np.float16)
        nf = npc // P
        xh[:, :nf * P] = shard[:nf * P].reshape(nf, P, P).transpose(1, 0, 2).reshape(P, nf * P)
        if npc % P:
            xh[0:npc % P, nf * P:(nf + 1) * P] = shard[nf * P:]
        in_maps.append({
            "x_hrm": xh, "x_tab": x16,
            "idx16": sched.idx16[c], "gmidx": sched.gm_idx16[c],
            "dstl": sched.dstl[c],
            "iota": iota, "ident": ident,
            "w1": w1, "w2": w2, "b1": b1r, "b2": b2r, "gam": gam, "bet": bet,
        })
    return in_maps


# ---------------------------------------------------------------------------
# PJRT runner: compile once, stage inputs on device, reuse the executable
# ---------------------------------------------------------------------------

def make_runner(nc, n_cores=NC):
    """Build a reusable jitted executable for the Bass module (axon/PJRT)."""
    import jax
    import numpy as _np
    from jax.sharding import Mesh, PartitionSpec
    from jax.experimental.shard_map import shard_map
    import concourse.bass2jax as b2j

    b2j.install_neuronx_cc_hook()
    partition_name = nc.partition_id_tensor.name if nc.partition_id_tensor else None
    in_names, out_names, out_avals, zero_shapes = [], [], [], []
    for alloc in nc.m.functions[0].allocations:
        if not isinstance(alloc, mybir.MemoryLocationSet):
            continue
        name = alloc.memorylocations[0].name
        if alloc.kind == "ExternalInput":
            if name != partition_name:
                in_names.append(name)
        elif alloc.kind == "ExternalOutput":
            out_names.append(name)
            shape = tuple(alloc.tensor_shape)
            dtype = mybir.dt.np(alloc.dtype)
            out_avals.append(jax.core.ShapedArray(shape, dtype))
            zero_shapes.append((shape, dtype))
    n_params = len(in_names)
    all_in = list(in_names) + list(out_names)
    if partition_name is not None:
        all_in.append(partition_name)

    def _body(*args):
        operands = list(args)
        if partition_name is not None:
            operands.append(b2j.partition_id_tensor())
        outs = b2j._bass_exec_p.bind(
            *operands,
            out_avals=tuple(out_avals),
            in_names=tuple(all_in),
            out_names=tuple(out_names),
            lowering_input_output_aliases=(),
            sim_require_finite=True,
            sim_require_nnan=True,
            nc=nc,
        )
        return tuple(outs)

    devices = jax.devices()[:n_cores]
    mesh = Mesh(_np.asarray(devices), ("core",))
    donate = tuple(range(n_params, n_params + len(out_names)))
    in_specs = (PartitionSpec("core"),) * (n_params + len(out_names))
    out_specs = (PartitionSpec("core"),) * len(out_names)
    sharded = jax.jit(
        shard_map(_body, mesh=mesh, in_specs=in_specs, out_specs=out_specs,
                  check_rep=False),
        donate_argnums=donate, keep_unused=True)
    return sharded, in_names, out_names, zero_shapes, mesh


_CACHE = {}


def _get_compiled(cfg, edge_index, batch):
    key = (cfg.N, cfg.E, cfg.L, cfg.G,
           hashlib.blake2b(np.ascontiguousarray(edge_index).tobytes(),
                           digest_size=16).hexdigest(),
           hashlib.blake2b(np.ascontiguousarray(batch).tobytes(),
                           digest_size=16).hexdigest())
    if key not in _CACHE:
        sched = build_schedule(cfg, edge_index, batch)
        nc = build_nc(cfg, sched)
        runner = make_runner(nc, NC)
        _CACHE[key] = (sched, nc, runner)
    return _CACHE[key]


def kernel(x, edge_index, batch, num_graphs, W1, b1, W2, b2, gamma, beta):
    """GIN forward on 8 TRN2 NeuronCores. Full inputs in, full output out."""
    import jax
    from jax.sharding import NamedSharding, PartitionSpec

    x = np.asarray(x, np.float32)
    edge_index = np.asarray(edge_index)
    batch = np.asarray(batch)
    G = int(np.asarray(num_graphs))
    cfg = Cfg(N=x.shape[0], E=edge_index.shape[1], L=np.asarray(W1).shape[0], G=G)

    sched, nc, (sharded, in_names, out_names, zero_shapes, mesh) = \
        _get_compiled(cfg, edge_index, batch)

    in_maps = prep_inputs(cfg, sched, x, W1, b1, W2, b2, gamma, beta)
    sh = NamedSharding(mesh, PartitionSpec("core"))
    concat_in = [np.concatenate([np.asarray(in_maps[c][n]) for c in range(NC)],
                                axis=0) for n in in_names]
    dev_in = [jax.device_put(a, sh) for a in concat_in]
    zeros = [jax.device_put(np.zeros((NC * s[0], *s[1:]), d), sh)
             for s, d in zero_shapes]
    outs = sharded(*dev_in, *zeros)
    gmaxT = np.asarray(outs[out_names.index("gmaxT")])  # [NC*P, G]
    out = np.ascontiguousarray(gmaxT[:P].T.astype(np.float32))
    # match jax segment_max: empty segments are -inf
    out[out <= -2.0e38] = -np.inf
    return out


# revision 13
# speedup vs baseline: 779.5537x; 1.0292x over previous
"""GIN (MoMuGNN) message-passing kernel for 8 TRN2 NeuronCores.

Full inputs in, full output out. All graph compute runs on device:
per-layer edge gather (SWDGE), one-hot scatter-add matmuls into PSUM,
MLP, batch-norm (stats via AllReduce), inter-layer fp16 AllGather of
node features, and the final per-graph segment-max (transpose-gather +
max reduces + AllReduce-max). Host work is limited to data layout
(transpose/cast/shard) and edge-schedule construction, cached per graph.

Edge gathers are descriptor-rate-bound (~8 ns per gathered row on the
GPSIMD SWDGE path), so the schedule minimizes gathered rows: edges are
chunked per (dst-core, 4-window group, src-half) bucket — group-level
chunking needs ~7% padding vs ~16% for per-window chunking — and each
128-edge chunk scatters via one narrow [128x128] one-hot matmul per
128-dst window segment it touches.
"""

import hashlib
import numpy as np
from dataclasses import dataclass, field

import concourse.tile as tile
from concourse import bacc, mybir

P = 128
NC = 8
BN_EPS = 1e-5
F32 = mybir.dt.float32
F16 = mybir.dt.float16


@dataclass
class Cfg:
    N: int
    E: int
    L: int
    G: int
    F: int = 128

    @property
    def npc(self):
        return self.N // NC

    @property
    def half(self):
        return self.N // 2

    @property
    def ntiles(self):
        return (self.npc + P - 1) // P

    def tsize(self, t):
        return min(P, self.npc - t * P)

    @property
    def groups(self):
        gs = []
        t = 0
        while t < self.ntiles:
            gs.append(list(range(t, min(t + 4, self.ntiles))))
            t += 4
        return gs


def _wrap_idx16(flat_idx: np.ndarray, nchunks: int) -> np.ndarray:
    """[nchunks*128] uint16 -> [128, nchunks*8] int16 in the wrapped layout
    dma_gather expects (16-partition wrap, replicated to 128)."""
    w16 = np.zeros((16, nchunks * 8), np.uint16)
    fi = flat_idx.reshape(nchunks * 8, 16)
    w16[:, :] = fi.T
    return np.tile(w16, (8, 1)).view(np.int16)


@dataclass
class Sched:
    K2: np.ndarray         # [ngroups, 2] chunks per (group, src-half), max over cores
    group_chunks: list     # per group: list (consumption order) of seg lists [(w, segcol)]
    nseg: int
    total_chunks: int
    idx16: list            # per core: [128, total_chunks*8] int16 wrapped
    dstl: list             # per core: [128, nseg] fp32 (window-local dst, -1 pad)
    # ---- segment-max schedule ----
    gm_reduces: list = field(default_factory=list)  # (graph, chunk_off, nchunks)
    gm_batches: list = field(default_factory=list)  # chunks per gather call
    gm_idx16: list = field(default_factory=list)    # per core: [128, gm_total*8]
    gm_total: int = 0


def build_schedule(cfg: Cfg, edge_index: np.ndarray, batch: np.ndarray) -> Sched:
    """Bucket edges per (dst-core, 4-window group, src-half); within a bucket
    edges are sorted by (window, src). Chunks are 128 edges; a chunk that
    straddles windows gets one one-hot segment per window (union over cores
    so the SPMD instruction stream is uniform; absent windows get all -1
    dst columns). Also builds the segment-max gather schedule."""
    src = edge_index[0].astype(np.int64)
    dst = edge_index[1].astype(np.int64)
    npc, half, ntiles = cfg.npc, cfg.half, cfg.ntiles
    groups = cfg.groups
    ngroups = len(groups)
    core = dst // npc
    loc = dst % npc
    wi = loc // P                  # window (=tile) within core
    dl = loc - wi * P              # dst local within window
    gi_of_w = np.zeros(ntiles, np.int64)
    for i, g in enumerate(groups):
        for w in g:
            gi_of_w[w] = i
    gidx = gi_of_w[wi]
    hf = (src >= half).astype(np.int64)

    order = np.lexsort((src, wi, hf, gidx, core))
    cs = core[order]
    gs_ = gidx[order]
    hs = hf[order]
    ws_ = wi[order]
    srcs = np.where(hs == 1, src[order] - half, src[order])
    dls = dl[order]
    key = (cs * ngroups + gs_) * 2 + hs
    bounds = np.searchsorted(key, np.arange(NC * ngroups * 2 + 1))
    buckets = {}
    cnt = np.zeros((NC, ngroups, 2), np.int64)
    for c in range(NC):
        for gi in range(ngroups):
            for h in range(2):
                k = (c * ngroups + gi) * 2 + h
                a, b = bounds[k], bounds[k + 1]
                buckets[(c, gi, h)] = (srcs[a:b], dls[a:b], ws_[a:b])
                cnt[c, gi, h] = b - a

    K2 = np.zeros((ngroups, 2), np.int64)
    for gi in range(ngroups):
        for h in range(2):
            m = cnt[:, gi, h].max()
            K2[gi, h] = (m + P - 1) // P if m > 0 else 0
        if K2[gi].sum() == 0:
            K2[gi, 0] = 1

    # segments: per (group, half, chunk) the union of windows over cores
    group_chunks = [[] for _ in range(ngroups)]
    seg_cols = 0
    chunk_cols = 0
    for gi in range(ngroups):
        for h in range(2):
            for j in range(int(K2[gi, h])):
                wins = set()
                for c in range(NC):
                    _s, _d, wv = buckets[(c, gi, h)]
                    seg = wv[j * P:(j + 1) * P]
                    if len(seg):
                        wins.update(np.unique(seg).tolist())
                if not wins:
                    wins = {groups[gi][0]}
                segs = [(int(w), seg_cols + k) for k, w in enumerate(sorted(wins))]
                seg_cols += len(segs)
                group_chunks[gi].append(segs)
                chunk_cols += 1
    nseg = seg_cols
    total_chunks = chunk_cols

    idx16, dstl = [], []
    for c in range(NC):
        flat_idx = np.zeros(total_chunks * P, np.uint16)
        flat_dl = np.full((P, nseg), -1.0, np.float32)
        pos = 0
        for gi in range(ngroups):
            ci = 0
            for h in range(2):
                s_arr, d_arr, w_arr = buckets[(c, gi, h)]
                n = len(s_arr)
                for j in range(int(K2[gi, h])):
                    rows = slice(j * P, (j + 1) * P)
                    sseg = s_arr[rows]
                    flat_idx[pos * P:pos * P + len(sseg)] = sseg.astype(np.uint16)
                    dseg = d_arr[rows]
                    wseg = w_arr[rows]
                    for (w, scol) in group_chunks[gi][ci]:
                        col = np.full(P, -1.0, np.float32)
                        m = wseg == w
                        col[:len(dseg)][m] = dseg[m].astype(np.float32)
                        flat_dl[:, scol] = col
                    pos += 1
                    ci += 1
        assert pos == total_chunks
        idx16.append(_wrap_idx16(flat_idx, total_chunks))
        dstl.append(flat_dl)

    # ---- segment-max gather schedule --------------------------------------
    # Per graph g, per core c: local node rows [a, b). Chunk count
    # C_g = max_c ceil(n_cg/128) (uniform). Cores pad with duplicates of a
    # local row of g, or the -inf sentinel row (npc) when they own none.
    batch = np.asarray(batch, np.int64)
    G = cfg.G
    starts = np.searchsorted(batch, np.arange(G))
    ends = np.searchsorted(batch, np.arange(1, G + 1))
    spans = []
    Cg = np.zeros(G, np.int64)
    for g in range(G):
        row = []
        for c in range(NC):
            c0, c1 = c * npc, (c + 1) * npc
            a, b = max(int(starts[g]), c0), min(int(ends[g]), c1)
            row.append((a - c0, b - c0) if b > a else (0, 0))
        spans.append(row)
        Cg[g] = max((b - a + P - 1) // P for a, b in row)

    gm_reduces = []
    off = 0
    for g in range(G):
        if Cg[g] > 0:
            gm_reduces.append((g, off, int(Cg[g])))
            off += int(Cg[g])
    gm_total = off

    gm_batches = []
    cur = 0
    for g, o, c in gm_reduces:
        if cur and cur + c > 64:
            gm_batches.append(cur)
            cur = 0
        cur += c
    if cur:
        gm_batches.append(cur)

    sentinel = npc  # z5T row holding -inf
    gm_idx16 = []
    for c in range(NC):
        flat = np.full(gm_total * P, sentinel, np.uint16)
        for g, o, nch in gm_reduces:
            a, b = spans[g][c]
            n = b - a
            if n == 0:
                continue
            ar = np.arange(o * P, o * P + nch * P)
            vals = np.full(nch * P, a, np.uint16)
            vals[:n] = np.arange(a, b, dtype=np.uint16)
            flat[ar] = vals
        gm_idx16.append(_wrap_idx16(flat, gm_total))

    return Sched(K2=K2, group_chunks=group_chunks, nseg=nseg,
                 total_chunks=total_chunks, idx16=idx16, dstl=dstl,
                 gm_reduces=gm_reduces, gm_batches=gm_batches,
                 gm_idx16=gm_idx16, gm_total=gm_total)


def build_nc(cfg: Cfg, sched: Sched, *, no_ar=False, no_ag=False,
             self_only=False, no_segmax=False):
    npc, ntiles, L, N, G = cfg.npc, cfg.ntiles, cfg.L, cfg.N, cfg.G
    half = cfg.half
    TC = sched.total_chunks
    NSEG = sched.nseg
    GMC = sched.gm_total
    K2 = sched.K2
    relu_op = mybir.ActivationFunctionType.Relu
    copy_op = mybir.ActivationFunctionType.Copy

    nc = bacc.Bacc("TRN2", target_bir_lowering=False, debug=False, num_devices=NC)

    xh_d = nc.dram_tensor("x_hrm", [P, ntiles * P], F16, kind="ExternalInput")
    xt_d = nc.dram_tensor("x_tab", [N, P], F16, kind="ExternalInput")
    idx_d = nc.dram_tensor("idx16", [P, TC * 8], mybir.dt.int16, kind="ExternalInput")
    gmidx_d = nc.dram_tensor("gmidx", [P, GMC * 8], mybir.dt.int16,
                             kind="ExternalInput")
    dstl_d = nc.dram_tensor("dstl", [P, NSEG], F32, kind="ExternalInput")
    iota_d = nc.dram_tensor("iota", [P, P], F32, kind="ExternalInput")
    ident_d = nc.dram_tensor("ident", [P, P], F32, kind="ExternalInput")
    w1_d = nc.dram_tensor("w1", [P, L * 2 * P], F32, kind="ExternalInput")
    w2_d = nc.dram_tensor("w2", [P, L * 2 * P], F32, kind="ExternalInput")
    b1_d = nc.dram_tensor("b1", [P, L * 2], F32, kind="ExternalInput")
    b2_d = nc.dram_tensor("b2", [P, L], F32, kind="ExternalInput")
    gam_d = nc.dram_tensor("gam", [P, L], F32, kind="ExternalInput")
    bet_d = nc.dram_tensor("bet", [P, L], F32, kind="ExternalInput")

    gmax_out = nc.dram_tensor("gmaxT", [P, G], F32, kind="ExternalOutput")

    ag_in = [nc.dram_tensor(f"ag_in_{l}", [npc, P], F16, kind="Internal")
             for l in range(L - 1)]
    ag_out = [nc.dram_tensor(f"ag_out_{l}", [N, P], F16, kind="Internal",
                             addr_space="Shared") for l in range(L - 1)]
    z5t_d = nc.dram_tensor("z5t", [npc + P, P], F16, kind="Internal")
    ar_in = [nc.dram_tensor(f"ar_in_{l}", [P, 2], F32, kind="Internal")
             for l in range(L)]
    ar_out = [nc.dram_tensor(f"ar_out_{l}", [P, 2], F32, kind="Internal",
                             addr_space="Shared") for l in range(L)]
    gm_in = nc.dram_tensor("gm_in", [P, G], F32, kind="Internal")
    gm_out = nc.dram_tensor("gm_out", [P, G], F32, kind="Internal",
                            addr_space="Shared")
    rg = [list(range(NC))]

    inv_n = 1.0 / N

    with tile.TileContext(nc) as tc:
        with tc.tile_pool(name="const", bufs=1) as cp, \
             tc.tile_pool(name="gath", bufs=3) as gp, \
             tc.tile_pool(name="oh", bufs=4) as ohp, \
             tc.tile_pool(name="zn", bufs=3) as znp, \
             tc.tile_pool(name="u", bufs=2) as up, \
             tc.tile_pool(name="small", bufs=8) as sp, \
             tc.tile_pool(name="scr", bufs=2) as scrp, \
             tc.tile_pool(name="gm", bufs=2) as gmp, \
             tc.tile_pool(name="ps_agg", bufs=2, space="PSUM") as pagg, \
             tc.tile_pool(name="ps_mlp", bufs=2, space="PSUM") as pmlp, \
             tc.tile_pool(name="ps_tp", bufs=2, space="PSUM") as ptp:

            # ---- persistent SBUF ----
            idx_sb = cp.tile([P, TC * 8], mybir.dt.int16)
            nc.sync.dma_start(out=idx_sb[:], in_=idx_d[:, :])
            gmidx_sb = cp.tile([P, GMC * 8], mybir.dt.int16)
            nc.sync.dma_start(out=gmidx_sb[:], in_=gmidx_d[:, :])
            dstl_sb = cp.tile([P, NSEG], F32)
            nc.sync.dma_start(out=dstl_sb[:], in_=dstl_d[:, :])
            iota_sb = cp.tile([P, P], F32)
            nc.sync.dma_start(out=iota_sb[:], in_=iota_d[:, :])
            ident_sb = cp.tile([P, P], F32)
            nc.sync.dma_start(out=ident_sb[:], in_=ident_d[:, :])
            w1_sb = cp.tile([P, L * 2 * P], F32)
            nc.sync.dma_start(out=w1_sb[:], in_=w1_d[:, :])
            w2_sb = cp.tile([P, L * 2 * P], F32)
            nc.sync.dma_start(out=w2_sb[:], in_=w2_d[:, :])
            b1_sb = cp.tile([P, L * 2], F32)
            nc.sync.dma_start(out=b1_sb[:], in_=b1_d[:, :])
            b2_sb = cp.tile([P, L], F32)
            nc.sync.dma_start(out=b2_sb[:], in_=b2_d[:, :])
            gam_sb = cp.tile([P, L], F32)
            nc.sync.dma_start(out=gam_sb[:], in_=gam_d[:, :])
            bet_sb = cp.tile([P, L], F32)
            nc.sync.dma_start(out=bet_sb[:], in_=bet_d[:, :])

            eps_sb = cp.tile([P, 1], F32)
            nc.vector.memset(eps_sb[:], BN_EPS)
            zero_sb = cp.tile([P, 1], F32)
            nc.vector.memset(zero_sb[:], 0.0)
            ninf_sb = cp.tile([P, P], F16)
            nc.vector.memset(ninf_sb[:], -60000.0)
            iota16 = cp.tile([P, P], F16)
            nc.vector.tensor_copy(out=iota16[:], in_=iota_sb[:])
            ident16 = cp.tile([P, P], F16)
            nc.vector.tensor_copy(out=ident16[:], in_=ident_sb[:])
            hrm = [cp.tile([P, ntiles * P], F16, name=f"hrm{i}") for i in range(2)]
            nc.sync.dma_start(out=hrm[1][:], in_=xh_d[:, :])
            z2all = cp.tile([P, npc], F32)
            nstats = len(cfg.groups)
            ssum = cp.tile([P, nstats], F32)
            ssq = cp.tile([P, nstats], F32)
            nc.sync.dma_start(out=z5t_d[npc:npc + P, :], in_=ninf_sb[:, :])

            ngroups = len(cfg.groups)
            group_cpos = []
            cpos = 0
            for gi in range(ngroups):
                group_cpos.append(cpos)
                cpos += int(K2[gi, 0] + K2[gi, 1])

            for l in range(L):
                table = xt_d if (l == 0 or no_ag) else ag_out[l - 1]
                selfbuf = hrm[(l - 1) % 2]
                last = l == L - 1

                for gi, g in enumerate(cfg.groups):
                    gw = sum(cfg.tsize(t) for t in g)
                    goff = g[0] * P
                    cp0 = group_cpos[gi]
                    klo = int(K2[gi, 0])
                    khi = int(K2[gi, 1])
                    kg = klo + khi
                    gt = gp.tile([P, kg * P], F16, name="gt", tag="gt")
                    if klo and not self_only:
                        nc.gpsimd.dma_gather(
                            gt[:, :klo * P].rearrange("p (c f) -> p c f", f=P),
                            table[0:half, :],
                            idx_sb[:, cp0 * 8:(cp0 + klo) * 8],
                            klo * P, klo * P, P, elem_step=P, single_packet=False)
                    if khi and not self_only:
                        nc.gpsimd.dma_gather(
                            gt[:, klo * P:kg * P].rearrange("p (c f) -> p c f", f=P),
                            table[half:N, :],
                            idx_sb[:, (cp0 + klo) * 8:(cp0 + kg) * 8],
                            khi * P, khi * P, P, elem_step=P, single_packet=False)

                    psum = pagg.tile([P, gw], F32, name="psum", tag="psum",
                                     padded_shape=[P, 4 * P], space="PSUM")
                    # one PSUM accumulation group: self matmuls first (start
                    # on the very first), then per-chunk window-segment
                    # matmuls, stop on the last segment.
                    toff = 0
                    for ti, t in enumerate(g):
                        ts_ = cfg.tsize(t)
                        nc.tensor.matmul(
                            out=psum[:, toff:toff + ts_],
                            lhsT=selfbuf[0:ts_, t * P:t * P + P],
                            rhs=ident16[0:ts_, 0:ts_],
                            start=(ti == 0),
                            stop=(self_only and ti == len(g) - 1))
                        toff += ts_
                    if not self_only:
                        chunks = sched.group_chunks[gi]
                        nmm = sum(len(c_) for c_ in chunks)
                        mm = 0
                        for j, segs in enumerate(chunks):
                            for (w, scol) in segs:
                                ts_ = cfg.tsize(w)
                                woff = (w - g[0]) * P
                                mm += 1
                                oh = ohp.tile([P, P], F16, name="oh", tag="oh")
                                nc.vector.tensor_scalar(
                                    out=oh[:, 0:ts_], in0=iota16[:, 0:ts_],
                                    scalar1=dstl_sb[:, scol:scol + 1],
                                    scalar2=None, op0=mybir.AluOpType.is_equal)
                                nc.tensor.matmul(
                                    out=psum[:, woff:woff + ts_],
                                    lhsT=gt[:, j * P:(j + 1) * P],
                                    rhs=oh[:, 0:ts_],
                                    start=False, stop=(mm == nmm))

                    # ---- MLP ----
                    zt = up.tile([P, gw], F32, name="zt", tag="zt",
                                 padded_shape=[P, 4 * P])
                    nc.scalar.activation(out=zt[:, :], in_=psum[:, :],
                                         func=copy_op, bias=0.0, scale=1.0)
                    u_t = [up.tile([P, gw], F32, name=f"u{hh}", tag=f"u{hh}",
                                   padded_shape=[P, 4 * P]) for hh in range(2)]
                    for hh in range(2):
                        ps1 = pmlp.tile([P, gw], F32, name="ps1", tag="ps1",
                                        padded_shape=[P, 4 * P], space="PSUM")
                        nc.tensor.matmul(
                            out=ps1[:, :],
                            lhsT=w1_sb[:, l * 2 * P + hh * P:l * 2 * P + hh * P + P],
                            rhs=zt[:, :],
                            start=True, stop=True)
                        nc.scalar.activation(
                            out=u_t[hh][:, :], in_=ps1[:, :], func=relu_op,
                            bias=b1_sb[:, l * 2 + hh:l * 2 + hh + 1], scale=1.0)
                    ps2 = pmlp.tile([P, gw], F32, name="ps2", tag="ps2",
                                    padded_shape=[P, 4 * P], space="PSUM")
                    for hh in range(2):
                        nc.tensor.matmul(
                            out=ps2[:, :],
                            lhsT=w2_sb[:, (l * 2 + hh) * P:(l * 2 + hh) * P + P],
                            rhs=u_t[hh][:, :],
                            start=(hh == 0), stop=(hh == 1))
                    nc.vector.tensor_scalar(
                        out=z2all[:, goff:goff + gw], in0=ps2[:, :],
                        scalar1=b2_sb[:, l:l + 1], scalar2=None,
                        op0=mybir.AluOpType.add)
                    nc.vector.tensor_reduce(
                        out=ssum[:, gi:gi + 1], in_=z2all[:, goff:goff + gw],
                        axis=mybir.AxisListType.X, op=mybir.AluOpType.add)
                    sq_scr = scrp.tile([P, 4 * P], F32, name="sq_scr", tag="sq")
                    nc.scalar.activation(
                        out=sq_scr[:, 0:gw], in_=z2all[:, goff:goff + gw],
                        func=mybir.ActivationFunctionType.Square,
                        bias=zero_sb[:, 0:1],
                        accum_out=ssq[:, gi:gi + 1])

                # ---- BN stats allreduce ----
                ar_sb = sp.tile([P, 2], F32, name="ar_sb", tag="ar")
                nc.vector.tensor_reduce(out=ar_sb[:, 0:1], in_=ssum[:, :],
                                        axis=mybir.AxisListType.X,
                                        op=mybir.AluOpType.add)
                nc.vector.tensor_reduce(out=ar_sb[:, 1:2], in_=ssq[:, :],
                                        axis=mybir.AxisListType.X,
                                        op=mybir.AluOpType.add)
                if no_ar:
                    arr = ar_sb
                else:
                    nc.sync.dma_start(out=ar_in[l][:, :], in_=ar_sb[:, :])
                    nc.gpsimd.collective_compute(
                        "AllReduce", mybir.AluOpType.add, replica_groups=rg,
                        ins=[ar_in[l][:, :]], outs=[ar_out[l][:, :]])
                    arr = sp.tile([P, 2], F32, name="arr", tag="ar")
                    nc.sync.dma_start(out=arr[:, :], in_=ar_out[l][:, :])

                stat = sp.tile([P, 6], F32, name="stat", tag="stat")
                mean, msq, var, istd, s_col, t_col = [stat[:, i:i + 1] for i in range(6)]
                nc.vector.tensor_scalar(out=mean, in0=arr[:, 0:1], scalar1=inv_n,
                                        scalar2=None, op0=mybir.AluOpType.mult)
                nc.vector.tensor_scalar(out=msq, in0=arr[:, 1:2], scalar1=inv_n,
                                        scalar2=None, op0=mybir.AluOpType.mult)
                sq_t = sp.tile([P, 2], F32, name="sq_t", tag="sq_t")
                nc.vector.tensor_tensor(out=sq_t[:, 0:1], in0=mean, in1=mean,
                                        op=mybir.AluOpType.mult)
                nc.vector.tensor_tensor(out=var, in0=msq, in1=sq_t[:, 0:1],
                                        op=mybir.AluOpType.subtract)
                std_t = sp.tile([P, 2], F32, name="std_t", tag="sq_t")
                nc.scalar.activation(out=std_t[:, 0:1], in_=var,
                                     func=mybir.ActivationFunctionType.Sqrt,
                                     bias=eps_sb[:, 0:1], scale=1.0)
                nc.vector.reciprocal(out=istd, in_=std_t[:, 0:1])
                nc.vector.tensor_tensor(out=s_col, in0=gam_sb[:, l:l + 1], in1=istd,
                                        op=mybir.AluOpType.mult)
                nc.vector.tensor_tensor(out=sq_t[:, 1:2], in0=mean, in1=s_col,
                                        op=mybir.AluOpType.mult)
                nc.vector.tensor_tensor(out=t_col, in0=bet_sb[:, l:l + 1],
                                        in1=sq_t[:, 1:2],
                                        op=mybir.AluOpType.subtract)

                # ---- normalize (+relu except last), transpose to row-major ----
                act = mybir.ActivationFunctionType.Identity if last else relu_op
                hout = hrm[l % 2]
                for t in range(ntiles):
                    ts_ = cfg.tsize(t)
                    zn = znp.tile([P, P], F16, name="zn16", tag="zn16")
                    nc.scalar.activation(out=zn[:, 0:ts_],
                                         in_=z2all[:, t * P:t * P + ts_],
                                         func=act, bias=t_col, scale=s_col)
                    tp = ptp.tile([P, P], F16, name="tp", tag="tp",
                                  space="PSUM")
                    nc.tensor.transpose(out=tp[0:ts_, :], in_=zn[:, 0:ts_],
                                        identity=ident16[:, :])
                    nc.vector.tensor_copy(out=hout[0:ts_, t * P:t * P + P],
                                          in_=tp[0:ts_, :])
                dst = z5t_d if last else ag_in[l]
                nfull = npc // P
                if nfull:
                    nc.sync.dma_start(
                        out=dst[0:nfull * P, :].rearrange("(t p) f -> p t f", p=P),
                        in_=hout[:, 0:nfull * P].rearrange("p (t f) -> p t f", f=P))
                if npc % P:
                    ts_ = npc % P
                    nc.sync.dma_start(
                        out=dst[nfull * P:npc, :],
                        in_=hout[0:ts_, nfull * P:nfull * P + P])
                if not last and not no_ag:
                    nc.gpsimd.collective_compute(
                        "AllGather", mybir.AluOpType.bypass, replica_groups=rg,
                        ins=[ag_in[l][:, :]], outs=[ag_out[l][:, :]])

            # ---- global max pool over graph ids (transpose-gather) ----
            gmax = cp.tile([P, G], F32)
            nc.vector.memset(gmax[:], -3.0e38)
            gm_reduces = [] if no_segmax else sched.gm_reduces
            gm_batches = [] if no_segmax else sched.gm_batches
            boff = 0
            bi = 0
            for nb in gm_batches:
                gat = gmp.tile([P, nb * P], F16, name="gmgat", tag="gmgat")
                nc.gpsimd.dma_gather(
                    gat[:, :].rearrange("p (o n) -> p o n", o=1),
                    z5t_d[0:npc + P, :],
                    gmidx_sb[:, boff * 8:(boff + nb) * 8],
                    nb * P, nb * P, P, elem_step=P, transpose=True,
                    single_packet=False)
                while bi < len(gm_reduces):
                    g, o, c = gm_reduces[bi]
                    if o + c > boff + nb:
                        break
                    nc.vector.tensor_reduce(
                        out=gmax[:, g:g + 1],
                        in_=gat[:, (o - boff) * P:(o - boff + c) * P],
                        axis=mybir.AxisListType.X, op=mybir.AluOpType.max)
                    bi += 1
                boff += nb
            assert bi == len(gm_reduces)
            if no_ar or no_segmax:
                nc.sync.dma_start(out=gmax_out[:, :], in_=gmax[:, :])
            else:
                nc.sync.dma_start(out=gm_in[:, :], in_=gmax[:, :])
                nc.gpsimd.collective_compute(
                    "AllReduce", mybir.AluOpType.max, replica_groups=rg,
                    ins=[gm_in[:, :]], outs=[gm_out[:, :]])
                gmax2 = sp.tile([P, G], F32, name="gmax2", tag="gmax2")
                nc.sync.dma_start(out=gmax2[:, :], in_=gm_out[:, :])
                nc.sync.dma_start(out=gmax_out[:, :], in_=gmax2[:, :])

    nc.compile()
    return nc


def prep_inputs(cfg: Cfg, sched: Sched, x, W1, b1, W2, b2, gamma, beta):
    """Per-core input maps. Host does data layout only: transpose/cast/shard."""
    N, L, ntiles, npc = cfg.N, cfg.L, cfg.ntiles, cfg.npc
    x = np.asarray(x, np.float32)
    x16 = np.ascontiguousarray(x.astype(np.float16))
    iota = np.broadcast_to(np.arange(P, dtype=np.float32), (P, P)).copy()
    ident = np.eye(P, dtype=np.float32)
    w1 = np.ascontiguousarray(np.transpose(np.asarray(W1, np.float32), (1, 0, 2))
                              ).reshape(P, L * 2 * P)
    w2 = np.ascontiguousarray(np.transpose(
        np.asarray(W2, np.float32).reshape(L, 2, P, P), (2, 0, 1, 3))
        ).reshape(P, L * 2 * P)
    b1r = np.ascontiguousarray(np.transpose(
        np.asarray(b1, np.float32).reshape(L, 2, P), (2, 0, 1))).reshape(P, L * 2)
    b2r = np.ascontiguousarray(np.asarray(b2, np.float32).T)
    gam = np.ascontiguousarray(np.asarray(gamma, np.float32).T)
    bet = np.ascontiguousarray(np.asarray(beta, np.float32).T)

    in_maps = []
    for c in range(NC):
        shard = x16[c * npc:(c + 1) * npc]
        xh = np.zeros((P, ntiles * P), np.float16)
        nf = npc // P
        xh[:, :nf * P] = shard[:nf * P].reshape(nf, P, P).transpose(1, 0, 2).reshape(P, nf * P)
        if npc % P:
            xh[0:npc % P, nf * P:(nf + 1) * P] = shard[nf * P:]
        in_maps.append({
            "x_hrm": xh, "x_tab": x16,
            "idx16": sched.idx16[c], "gmidx": sched.gm_idx16[c],
            "dstl": sched.dstl[c],
            "iota": iota, "ident": ident,
            "w1": w1, "w2": w2, "b1": b1r, "b2": b2r, "gam": gam, "bet": bet,
        })
    return in_maps


# ---------------------------------------------------------------------------
# PJRT runner: compile once, stage inputs on device, reuse the executable
# ---------------------------------------------------------------------------

def make_runner(nc, n_cores=NC):
    """Build a reusable jitted executable for the Bass module (axon/PJRT)."""
    import jax
    import numpy as _np
    from jax.sharding import Mesh, PartitionSpec
    from jax.experimental.shard_map import shard_map
    import concourse.bass2jax as b2j

    b2j.install_neuronx_cc_hook()
    partition_name = nc.partition_id_tensor.name if nc.partition_id_tensor else None
    in_names, out_names, out_avals, zero_shapes = [], [], [], []
    for alloc in nc.m.functions[0].allocations:
        if not isinstance(alloc, mybir.MemoryLocationSet):
            continue
        name = alloc.memorylocations[0].name
        if alloc.kind == "ExternalInput":
            if name != partition_name:
                in_names.append(name)
        elif alloc.kind == "ExternalOutput":
            out_names.append(name)
            shape = tuple(alloc.tensor_shape)
            dtype = mybir.dt.np(alloc.dtype)
            out_avals.append(jax.core.ShapedArray(shape, dtype))
            zero_shapes.append((shape, dtype))
    n_params = len(in_names)
    all_in = list(in_names) + list(out_names)
    if partition_name is not None:
        all_in.append(partition_name)

    def _body(*args):
        operands = list(args)
        if partition_name is not None:
            operands.append(b2j.partition_id_tensor())
        outs = b2j._bass_exec_p.bind(
            *operands,
            out_avals=tuple(out_avals),
            in_names=tuple(all_in),
            out_names=tuple(out_names),
            lowering_input_output_aliases=(),
            sim_require_finite=True,
            sim_require_nnan=True,
            nc=nc,
        )
        return tuple(outs)

    devices = jax.devices()[:n_cores]
    mesh = Mesh(_np.asarray(devices), ("core",))
    donate = tuple(range(n_params, n_params + len(out_names)))
    in_specs = (PartitionSpec("core"),) * (n_params + len(out_names))
    out_specs = (PartitionSpec("core"),) * len(out_names)
    sharded = jax.jit(
        shard_map(_body, mesh=mesh, in_specs=in_specs, out_specs=out_specs,
                  check_rep=False),
        donate_argnums=donate, keep_unused=True)
    return sharded, in_names, out_names, zero_shapes, mesh


_CACHE = {}


def _get_compiled(cfg, edge_index, batch):
    key = (cfg.N, cfg.E, cfg.L, cfg.G,
           hashlib.blake2b(np.ascontiguousarray(edge_index).tobytes(),
                           digest_size=16).hexdigest(),
           hashlib.blake2b(np.ascontiguousarray(batch).tobytes(),
                           digest_size=16).hexdigest())
    if key not in _CACHE:
        sched = build_schedule(cfg, edge_index, batch)
        nc = build_nc(cfg, sched)
        runner = make_runner(nc, NC)
        _CACHE[key] = (sched, nc, runner)
    return _CACHE[key]


def kernel(x, edge_index, batch, num_graphs, W1, b1, W2, b2, gamma, beta):
    """GIN forward on 8 TRN2 NeuronCores. Full inputs in, full output out."""
    import jax
    from jax.sharding import NamedSharding, PartitionSpec

    x = np.asarray(x, np.float32)
    edge_index = np.asarray(edge_index)
    batch = np.asarray(batch)
    G = int(np.asarray(num_graphs))
    cfg = Cfg(N=x.shape[0], E=edge_index.shape[1], L=np.asarray(W1).shape[0], G=G)

    sched, nc, (sharded, in_names, out_names, zero_shapes, mesh) = \
        _get_compiled(cfg, edge_index, batch)

    in_maps = prep_inputs(cfg, sched, x, W1, b1, W2, b2, gamma, beta)
    sh = NamedSharding(mesh, PartitionSpec("core"))
    concat_in = [np.concatenate([np.asarray(in_maps[c][n]) for c in range(NC)],
                                axis=0) for n in in_names]
    dev_in = [jax.device_put(a, sh) for a in concat_in]
    zeros = [jax.device_put(np.zeros((NC * s[0], *s[1:]), d), sh)
             for s, d in zero_shapes]
    outs = sharded(*dev_in, *zeros)
    gmaxT = np.asarray(outs[out_names.index("gmaxT")])  # [NC*P, G]
    out = np.ascontiguousarray(gmaxT[:P].T.astype(np.float32))
    # match jax segment_max: empty segments are -inf
    out[out <= -2.0e38] = -np.inf
    return out
